# revision 44
# baseline (speedup 1.0000x reference)
"""Trainium2 Bass kernel for nn_BidirectionalAttention (B=2, N=2048, D=2048, H=16).

Head-parallel tensor sharding across 8 NeuronCores (2 heads/core), all-bf16
matmul pipeline (fp32 PSUM accumulation):

  warmup:  dummy matmuls with no DMA deps run from t=0 (plus pads woven into
           the DMA-paced first groups) so the PE HAM clock-gate opens (K=8/8)
           before the first real matmul and never re-throttles.
  phase A: weight-stationary qkv projection: q^T/k^T [dh, tok] come straight
           out of PSUM (no PE transposes). Rope applied in transposed layout:
           ScalarE copies q^T PSUM->SBUF (bf16); the rotate-half partner sits
           16 partitions away inside the same 32-partition quadrant (host-side
           dh permutation), so a DVE stream_shuffle produces it and the sign
           is folded into the sin table. v computed x-stationary in natural
           [tok, dh] layout and kept SBUF-resident.
  phase B: per (batch, qtile, head): scores s^T[k,q] = k^T.T @ q^T, exp on
           ScalarE (bf16 out), level-1 pair-adds split DVE/GpSimd, rest of
           the reduction tree on DVE.  The attn@v matmuls for a qtile are
           DEFERRED one slot so the PE never waits on the exp chain: slot i
           runs scores(i) | av(i-1) | bcast(i-1) | C subtiles while exp(i)
           proceeds on ScalarE.
  phase C: output projection partial = av^T.T @ wo_rows per core, emitted as
           2 subtiles per slot as soon as both heads' av_r for a qtile exist
           (qt-major head ordering makes that early), bf16 partials.
Host: shard/transpose inputs to bf16, sum the 8 partial outputs in f32 (the
"all-reduce after wo" done at gather time).
"""

import os
import sys

sys.path.insert(0, "/opt/trn_rl_repo")

import numpy as np
import ml_dtypes

B, SEQ, DIM, NHEAD, DH = 2, 2048, 2048, 16, 128
HL = NHEAD // 8  # heads per core = 2
NCORES = 8
NT = B * SEQ  # 4096 flattened rows
SCALE = 1.0 / np.sqrt(DH)

_PROG = {}

# rotate-half partner lives 16 partitions away within each 32-part quadrant
SHUF_MASK = [(i + 16) % 32 for i in range(32)]


def _build(dt_name: str):
    import concourse.tile as tile
    from concourse import bacc, mybir

    f32 = mybir.dt.float32
    dmm = {"bf16": mybir.dt.bfloat16, "f32r": mybir.dt.float32r}[dt_name]
    Exp = mybir.ActivationFunctionType.Exp

    nc = bacc.Bacc("TRN2", target_bir_lowering=False, debug=False, num_devices=NCORES)

    xt_d = nc.dram_tensor("xt", [DIM, NT], dmm, kind="ExternalInput")
    wqk_d = nc.dram_tensor("wqk", [DIM, 4 * DH], dmm, kind="ExternalInput")
    wv_d = nc.dram_tensor("wv", [DIM, HL * DH], dmm, kind="ExternalInput")
    wo_d = nc.dram_tensor("wo_r", [HL * DH, DIM], dmm, kind="ExternalInput")
    cos_d = nc.dram_tensor("cosd", [128, NT], dmm, kind="ExternalInput")
    sin_d = nc.dram_tensor("sind", [128, NT], dmm, kind="ExternalInput")
    onesf_d = nc.dram_tensor("onesf", [128, 128], dmm, kind="ExternalInput")
    out_d = nc.dram_tensor("out_p", [NT, DIM], dmm, kind="ExternalOutput")

    with tile.TileContext(nc) as tc:
        with (
            nc.allow_low_precision(reason="bf16 matmul pipeline"),
            tc.tile_pool(name="const", bufs=1) as cp,
        ):
            onesf = cp.tile([128, 128], dmm)
            # q^T / k^T SBUF-resident across phases: [tensor t][128 dh, NT]
            qkt_res = [
                cp.tile([128, NT], dmm, name=f"qktres{t}", tag=f"qktres{t}")
                for t in range(4)
            ]
            # v natural layout, SBUF-resident: [128 tok%128, 32 tile, 2*DH]
            v_all = cp.tile([128, NT // 128, HL * DH], dmm, name="vall", tag="vall")
            # zero tile for warmup/padding matmuls (no DMA dependency): the
            # PE HAM clock-gate throttles to half clock after any >3.4us PE
            # idle window, so dummy matmuls pad the DMA-paced stretches.
            wsb = cp.tile([128, 512], dmm, name="warmsb", tag="warmsb")
            nc.vector.memset(wsb, 0.0)
            # last group's x snapshot + wv stay resident: its v-projection is
            # deferred into the first phase-B slots (which otherwise idle
            # waiting on the exp chain)
            xg7 = cp.tile([128, 16, 512], dmm, name="xg7", tag="xg7")
            wv_sb = cp.tile([128, 16, HL * DH], dmm, name="wvsb", tag="wvsb")

            # ---------------- Phase A: qkv projection + rope ----------------
            GW = 512  # tokens per group
            NG = NT // GW  # 8 groups
            with (
                tc.tile_pool(name="aconst", bufs=1) as ac,
                tc.tile_pool(name="axs", bufs=2) as axs,
                tc.tile_pool(name="awork", bufs=3) as aw,
                tc.tile_pool(name="aqk", bufs=4, space="PSUM") as aqk,
                tc.tile_pool(name="avp", bufs=2, space="PSUM") as avp,
                tc.tile_pool(name="apad", bufs=2, space="PSUM") as apad,
            ):
                def pad_mm(n):
                    # PE busy-work with no DMA deps: fills DMA-paced gaps so
                    # the HAM clock-gate never sees a >3.4us idle window
                    for _ in range(n):
                        pps = apad.tile([128, 512], f32, tag="pad", name="padps")
                        nc.tensor.matmul(pps, wsb[:, 0:128], wsb, start=True, stop=True)

                # warmup until the first real matmul's data lands (~18us per
                # trace: NEFF preamble + DMA ring startup + first chunks)
                pad_mm(16)

                wqk_sb = ac.tile([128, 16, 4 * DH], dmm)
                wqk_src = wqk_d.rearrange("(c p) m -> p c m", p=128)
                wv_src = wv_d.rearrange("(c p) m -> p c m", p=128)
                cos_sb = ac.tile([128, NT], dmm)
                sin_sb = ac.tile([128, NT], dmm)
                xt_all = xt_d.rearrange("(c p) n -> p c n", p=128)
                # Each DMA queue sustains only ~120 GB/s, so the x stream is
                # striped across all three queues (sync / scalar / gpsimd).
                # Upfront: sync carries wqk (chunk-interleaved with xs0 use),
                # scalar carries xs0, gpsimd carries cos/sin g0 + xs1 + wv.
                xs0 = axs.tile([128, 16, GW], dmm, tag="xs", bufs=4)
                for cc in range(16):
                    nc.sync.dma_start(wqk_sb[:, cc, :], wqk_src[:, cc, :])
                    nc.scalar.dma_start(xs0[:, cc, :], xt_all[:, cc, 0:GW])
                nc.gpsimd.dma_start(cos_sb[:, 0:GW], cos_d[:, 0:GW])
                nc.gpsimd.dma_start(sin_sb[:, 0:GW], sin_d[:, 0:GW])

                xs_pend = {0: xs0}

                def load_xs(g, eng):
                    xs = axs.tile([128, 16, GW], dmm, tag="xs", bufs=4)
                    xt_src = xt_all[:, :, g * GW : (g + 1) * GW]
                    for cg in range(4):
                        eng.dma_start(
                            xs[:, 4 * cg : 4 * cg + 4, :],
                            xt_src[:, 4 * cg : 4 * cg + 4, :],
                        )
                    xs_pend[g] = xs

                load_xs(1, nc.gpsimd)
                for cc in range(4):
                    nc.gpsimd.dma_start(
                        wv_sb[:, 4 * cc : 4 * cc + 4, :],
                        wv_src[:, 4 * cc : 4 * cc + 4, :],
                    )
                nc.sync.dma_start(onesf, onesf_d[:, :])
                nc.gpsimd.dma_start(cos_sb[:, GW : 2 * GW], cos_d[:, GW : 2 * GW])
                nc.gpsimd.dma_start(sin_sb[:, GW : 2 * GW], sin_d[:, GW : 2 * GW])
                load_xs(2, nc.sync)
                for g in range(NG):
                    g0 = g * GW
                    if g + 2 < NG:
                        # later rope tables on sync (small, ample slack)
                        nl = slice((g + 2) * GW, (g + 3) * GW)
                        nc.sync.dma_start(cos_sb[:, nl], cos_d[:, nl])
                        nc.sync.dma_start(sin_sb[:, nl], sin_d[:, nl])
                    if g + 3 < NG:
                        # in-loop emission keeps each trigger's WAR (xs tile
                        # reuse) already resolved so it never blocks the
                        # issuing engine's queue
                        load_xs(g + 3, [nc.scalar, nc.gpsimd, nc.sync][g % 3])
                    xs = xs_pend.pop(g)
                    # q0 q1 k0 k1 (dh on partitions), weight-stationary.
                    tmps = []
                    for t in range(4):
                        qps = aqk.tile([128, GW], f32, tag="qk", name=f"qps{t}")
                        for cc in range(16):
                            nc.tensor.matmul(
                                qps,
                                wqk_sb[:, cc, t * 128 : (t + 1) * 128],
                                xs[:, cc, :],
                                start=(cc == 0),
                                stop=(cc == 15),
                            )
                            # first groups' first tensor is paced by chunk
                            # arrival: pad the gaps so HAM stays warm
                            if g == 0 and t == 0 and cc < 15:
                                pad_mm(2 if cc % 2 == 1 else 1)
                            elif g == 1 and t == 0 and cc < 15:
                                pad_mm(1)
                        tmp = aw.tile([128, GW], dmm, tag="tmp", bufs=5)
                        nc.scalar.copy(tmp, qps)
                        tmps.append(tmp)
                    for t in range(4):
                        tmp = tmps[t]
                        # rotate-half on DVE: partner is 16 partitions away in
                        # the same quadrant; sign is folded into sind
                        rsh = aw.tile([128, GW], dmm, tag="rsh", bufs=2)
                        nc.vector.stream_shuffle(rsh, tmp, SHUF_MASK)
                        m1 = aw.tile([128, GW], dmm, tag="m1", bufs=2)
                        nc.vector.tensor_mul(m1, tmp, cos_sb[:, g0 : g0 + GW])
                        m2 = aw.tile([128, GW], dmm, tag="m2", bufs=2)
                        nc.vector.tensor_mul(m2, rsh, sin_sb[:, g0 : g0 + GW])
                        nc.vector.tensor_add(qkt_res[t][:, g0 : g0 + GW], m1, m2)
                    if g == NG - 1:
                        # defer this group's v-projection into phase-B slot
                        # fillers; snapshot xs (its pool closes before B)
                        nc.vector.tensor_copy(xg7, xs)
                        continue
                    # v natural layout, x-stationary
                    for st in range(GW // 128):
                        vps = avp.tile([128, HL * DH], f32, tag="v")
                        for cc in range(16):
                            nc.tensor.matmul(
                                vps,
                                xs[:, cc, st * 128 : (st + 1) * 128],
                                wv_sb[:, cc, :],
                                start=(cc == 0),
                                stop=(cc == 15),
                            )
                        nc.scalar.copy(v_all[:, g * (GW // 128) + st, :], vps)
                        if g == 0:
                            pad_mm(1)

            # ---------- Phase B+C: attention + output projection ------------
            # Slot structure: slot i emits scores(i) score-pairs with "filler"
            # work WOVEN between them -- av matmuls + softmax-scale for slot
            # i-1 and pending C subtiles -- so the in-order PE always has
            # independent work while the serial exp chain (the slot's pacing
            # resource on ScalarE) drains the scores PSUM banks.
            with (
                tc.tile_pool(name="bprobs", bufs=3) as bp,
                tc.tile_pool(name="btree", bufs=2) as btr,
                tc.tile_pool(name="bwork", bufs=3) as bw,
                tc.tile_pool(name="bavres", bufs=4) as bav_sb,
                tc.tile_pool(name="cot", bufs=4) as cot,
                tc.tile_pool(name="bs", bufs=2, space="PSUM") as bs,
                tc.tile_pool(name="bavp", bufs=1, space="PSUM") as bavp,
                tc.tile_pool(name="bops", bufs=3, space="PSUM") as bops,
            ):
                wo_sb = bp.tile([128, HL, DIM], dmm, tag="wo", bufs=1)
                nc.sync.dma_start(wo_sb, wo_d.rearrange("(j p) o -> p j o", p=128))
                avres = {}

                def make_fin_units(st):
                    # filler closures: av matmuls + softmax scale for a
                    # previous slot. Order matters: av chunks first, then
                    # bcast+recip, then the final mul.
                    b, j, q0, probs, sumb, av_r = st
                    avps_box, rbc_box = [], []

                    def av_chunk(c4):
                        def go():
                            if not avps_box:
                                avps_box.append(
                                    bavp.tile([128, 512], f32, tag="avp", name="avps")
                                )
                            for cc in range(4 * c4, 4 * c4 + 4):
                                nc.tensor.matmul(
                                    avps_box[0],
                                    v_all[:, b * 16 + cc, j * DH : (j + 1) * DH],
                                    probs[:, cc, :],
                                    start=(cc == 0),
                                    stop=(cc == 15),
                                )

                        return go

                    def bcast_recip():
                        # ones stationary: sums across partitions AND
                        # broadcasts the result to all 128 partitions
                        rbc_ps = bops.tile([128, 512], f32, tag="ops", name="rbc_ps")
                        nc.tensor.matmul(rbc_ps, onesf, sumb, start=True, stop=True)
                        rbc = bw.tile([128, 512], f32, tag="rbcsb", name="rbc")
                        nc.vector.reciprocal_approx_fast(rbc, rbc_ps)
                        rbc_box.append(rbc)

                    def fin_mul():
                        nc.vector.tensor_mul(
                            av_r[:, q0 : q0 + 512], avps_box[0], rbc_box[0]
                        )

                    return ([av_chunk(0), av_chunk(1), av_chunk(2), av_chunk(3)],
                            bcast_recip, fin_mul)

                def make_c_units(b, nl, tail=False):
                    g0 = b * SEQ + nl * 128
                    ot_box = []

                    def c_ops(do):
                        def go():
                            if not ot_box:
                                ot_box.append(
                                    cot.tile([128, DIM], dmm, tag="ot", name="ot")
                                )
                            ot = ot_box[0]
                            ops = bops.tile([128, 512], f32, tag="ops", name="ops")
                            for j in range(HL):
                                nc.tensor.matmul(
                                    ops,
                                    avres[(b, j)][:, nl * 128 : (nl + 1) * 128],
                                    wo_sb[:, j, do * 512 : (do + 1) * 512],
                                    start=(j == 0),
                                    stop=(j == 1),
                                )
                            osl = ot[:, do * 512 : (do + 1) * 512]
                            # one copy per subtile on ScalarE (fits after the
                            # exp chain), rest on DVE; tail flush splits half
                            if do == 0 or (tail and do == 2):
                                nc.scalar.copy(osl, ops)
                            else:
                                nc.vector.tensor_copy(osl, ops)
                            if tail:
                                nc.sync.dma_start(
                                    out_d[g0 : g0 + 128, do * 512 : (do + 1) * 512],
                                    osl,
                                )
                            elif do == 3:
                                nc.sync.dma_start(out_d[g0 : g0 + 128, :], ot)

                        return go

                    return [c_ops(0), c_ops(1), c_ops(2), c_ops(3)]

                def emit_slot(b, j, qt_i, av_r, fillers):
                    kt_sb = qkt_res[2 + j][:, b * SEQ : (b + 1) * SEQ]
                    qt_sb = qkt_res[j][:, b * SEQ : (b + 1) * SEQ]
                    q0 = qt_i * 512
                    probs = bp.tile([128, 16, 512], dmm, tag="probs", name="probs")
                    ps8 = btr.tile([128, 8, 512], dmm, tag="ps8", name="ps8")
                    fi, nf = 0, len(fillers)
                    for kp in range(8):
                        sps = bs.tile([128, 2, 512], f32, tag="s", name="sps")
                        for u in range(2):
                            kt_i = 2 * kp + u
                            nc.tensor.matmul(
                                sps[:, u, :],
                                kt_sb[:, kt_i * 128 : (kt_i + 1) * 128],
                                qt_sb[:, q0 : q0 + 512],
                                start=True,
                                stop=True,
                            )
                        nc.scalar.activation(probs[:, 2 * kp : 2 * kp + 2, :], sps, Exp)
                        # level-1 pair-add: mostly on GpSimd (ScalarE must
                        # stay exp-only and DVE carries the C casts); the
                        # last pairs on DVE so the tree isn't gated on the
                        # slower GpSimd chain
                        eng = nc.gpsimd if kp < 5 else nc.vector
                        eng.tensor_add(
                            ps8[:, kp, :], probs[:, 2 * kp, :], probs[:, 2 * kp + 1, :]
                        )
                        for _ in range(1 if kp < 4 else 2):
                            if fi < nf:
                                fillers[fi]()
                                fi += 1
                    while fi < nf:
                        fillers[fi]()
                        fi += 1
                    # rest of the k-tile reduction (slice-halving); sumb is
                    # consumed by next slot's bcast -> ample slack
                    tb = btr.tile([128, 4, 512], dmm, tag="tb", name="tb")
                    nc.vector.tensor_add(tb, ps8[:, 0:4, :], ps8[:, 4:8, :])
                    tc2 = btr.tile([128, 2, 512], dmm, tag="tc", name="tc2")
                    nc.vector.tensor_add(tc2, tb[:, 0:2, :], tb[:, 2:4, :])
                    sumb = btr.tile([128, 512], dmm, tag="sumb", name="sumb")
                    nc.vector.tensor_add(sumb, tc2[:, 0, :], tc2[:, 1, :])
                    return (b, j, q0, probs, sumb, av_r)

                def dummy_unit():
                    # PE busy-work for the filler-poor first slots: stops the
                    # exp-paced gaps from re-throttling the HAM clock-gate
                    dps = bops.tile([128, 512], f32, tag="ops", name="dps")
                    for _ in range(2):
                        nc.tensor.matmul(dps, wsb[:, 0:128], wsb, start=True, stop=True)

                def v_unit(u):
                    # deferred v-projection of the last phase-A group: real
                    # PE filler for the first slots (two subtiles packed into
                    # one PSUM tile, halves written back separately)
                    def go():
                        vtile = bops.tile([128, 512], f32, tag="ops", name="vps7")
                        for st in range(2):
                            s = 2 * u + st
                            for cc in range(16):
                                nc.tensor.matmul(
                                    vtile[:, st * 256 : (st + 1) * 256],
                                    xg7[:, cc, s * 128 : (s + 1) * 128],
                                    wv_sb[:, cc, :],
                                    start=(cc == 0),
                                    stop=(cc == 15),
                                )
                            nc.scalar.copy(
                                v_all[:, 28 + s, :],
                                vtile[:, st * 256 : (st + 1) * 256],
                            )
                    return go

                # qt-major over heads so both heads' av_r for a qtile complete
                # early and C work spreads evenly across slots
                pend = None
                c_queue = []  # pending C subtiles (b, nl)
                slot_idx = 0
                for b in range(B):
                    for j in range(HL):
                        avres[(b, j)] = bav_sb.tile(
                            [128, SEQ], dmm, tag="avres", name="av_r"
                        )
                    for qt_i in range(4):
                        for j in range(HL):
                            c_units = []
                            for _ in range(2):
                                if c_queue:
                                    c_units += make_c_units(*c_queue.pop(0))
                            if pend is not None:
                                # bcast sits late (~kp6 PE position) so the
                                # previous slot's DVE tree has time to land;
                                # mul after it (needs rbc + all av chunks)
                                avu, bcast_u, mul_u = make_fin_units(pend)
                                fillers = (
                                    avu
                                    + c_units[0:2]
                                    + [bcast_u]
                                    + c_units[2:4]
                                    + [mul_u]
                                    + c_units[4:]
                                )
                            else:
                                fillers = c_units
                            if slot_idx == 0:
                                fillers += [v_unit(0), dummy_unit, dummy_unit]
                            elif slot_idx == 1:
                                fillers += [v_unit(1)]
                            elif slot_idx == 2:
                                fillers += [dummy_unit] * 2
                            # C for a qtile becomes available one slot AFTER
                            # its j==1 fin is emitted (fin's mul must land)
                            if pend is not None and pend[1] == 1:
                                pb, pq0 = pend[0], pend[2]
                                c_queue += [(pb, pq0 // 128 + u) for u in range(4)]
                            pend = emit_slot(b, j, qt_i, avres[(b, j)], fillers)
                            slot_idx += 1
                # flush: final fin + remaining C subtiles
                avu, bcast_u, mul_u = make_fin_units(pend)
                for f in avu + [bcast_u, mul_u]:
                    f()
                c_queue += [(B - 1, 12 + u) for u in range(4)]
                while c_queue:
                    bb, nl = c_queue.pop(0)
                    for f in make_c_units(bb, nl, tail=(len(c_queue) < 2)):
                        f()

    nc.compile()
    return nc


def _get_prog():
    dt_name = os.environ.get("KMM_DT", "bf16")
    key = ("prog", dt_name)
    if key not in _PROG:
        _PROG[key] = _build(dt_name)
    return _PROG[key], dt_name


def _shard(x, freqs_cis, wqkv, wo, dt_name):
    np_dt = {"bf16": ml_dtypes.bfloat16, "f32r": np.float32}[dt_name]

    def rnd(a):
        return np.ascontiguousarray(a, dtype=np.float32).astype(np_dt)

    x = np.asarray(x, dtype=np.float32)
    freqs_cis = np.asarray(freqs_cis, dtype=np.float32)
    wqkv = np.asarray(wqkv, dtype=np.float32)
    wo = np.asarray(wo, dtype=np.float32)

    xt = rnd(x.reshape(NT, DIM).T)

    # dh permutation: partition p = 32*quad + jj holds component
    #   jj < 16  -> even component of freq f = 16*quad + jj      (sign -1)
    #   jj >= 16 -> odd  component of freq f = 16*quad + jj - 16 (sign +1)
    # so the rotate-half partner is 16 partitions away inside the quadrant
    # and stream_shuffle(mask=(i+16)%32) produces it; the sign lives in sind.
    quad = np.arange(128) // 32
    jj = np.arange(128) % 32
    freq = 16 * quad + (jj % 16)
    sign = np.where(jj < 16, -1.0, 1.0).astype(np.float32)
    perm = 2 * freq + (jj >= 16)  # source component index in natural dh

    cos = freqs_cis[:, :, 0].T  # [64, SEQ] per-freq
    sin = freqs_cis[:, :, 1].T
    cosb = np.concatenate([cos] * B, axis=1)  # [64, NT]
    sinb = np.concatenate([sin] * B, axis=1)
    cosd = rnd(cosb[freq])  # [128, NT]
    sind = rnd(sinb[freq] * sign[:, None])

    consts = {"onesf": rnd(np.ones((128, 128), np.float32))}
    in_maps = []
    for c in range(NCORES):
        h0 = c * HL
        wq = [wqkv[:, h * DH : (h + 1) * DH][:, perm] * SCALE for h in (h0, h0 + 1)]
        wk = [wqkv[:, DIM + h * DH : DIM + (h + 1) * DH][:, perm] for h in (h0, h0 + 1)]
        wqk_c = rnd(np.concatenate(wq + wk, axis=1))  # [DIM, 512]
        wv_c = rnd(wqkv[:, 2 * DIM + h0 * DH : 2 * DIM + (h0 + HL) * DH])  # [DIM, 256]
        wo_c = rnd(wo[h0 * DH : (h0 + HL) * DH, :])  # [256, DIM]
        in_maps.append(
            {
                "xt": xt,
                "wqk": wqk_c,
                "wv": wv_c,
                "wo_r": wo_c,
                "cosd": cosd,
                "sind": sind,
                **consts,
            }
        )
    return in_maps


def _run(in_maps, trace=False, **kw):
    from concourse.bass_utils import run_bass_kernel_spmd

    prog, _ = _get_prog()
    return run_bass_kernel_spmd(prog, in_maps, list(range(NCORES)), trace=trace, **kw)


def kernel(x, freqs_cis, wqkv, wo):
    _, dt_name = _get_prog()
    in_maps = _shard(x, freqs_cis, wqkv, wo, dt_name)
    res = _run(in_maps, trace=False)
    acc = np.zeros((NT, DIM), dtype=np.float32)
    for c in range(NCORES):
        acc += np.asarray(res.results[c]["out_p"]).astype(np.float32)
    return acc.reshape(B, SEQ, DIM)
